# revision 2
# baseline (speedup 1.0000x reference)
"""Trainium2 Bass kernel for nn_CGCNN (gnn_message_passing), 8 NeuronCores.

Self-contained: takes full unsharded inputs, shards nodes/edges across 8
cores internally (node-relabeled, in-degree-balanced 128-node blocks; edges
assigned to the dst-owning core), runs a Bass/Tile kernel via
run_bass_kernel_spmd, and returns the full [256] output.
"""
import os, sys
for _p in ("/opt/trn_rl_repo", "/root/.axon_site/_ro/trn_rl_repo"):
    if os.path.isdir(_p) and _p not in sys.path:
        sys.path.append(_p)
import numpy as np
import concourse.bacc as bacc
import concourse.bass as bass
import concourse.mybir as mybir
import concourse.tile as tile
from concourse import library_config
from concourse.masks import make_identity
from concourse.bass_utils import run_bass_kernel_spmd

P = 128
F32 = mybir.dt.float32
I16 = mybir.dt.int16
AF = mybir.ActivationFunctionType
OP = mybir.AluOpType

N_CORES = 8
NG = 256          # graph slots (padded)
D_HID = 256

N_NODES = 25000
N_GRAPHS = 256










def softplus(x):
    return np.logaddexp(0.0, x)


def prep(inputs, n_nodes, n_graphs, blocks_per_core=25):
    """Build all per-core device arrays. Returns (cfg, per_core list, consts)."""
    x_atoms = np.asarray(inputs["x_atoms"]).astype(np.int64)
    edge_index = np.asarray(inputs["edge_index"]).astype(np.int64)
    edge_attr = np.asarray(inputs["edge_attr"]).astype(np.float32)
    batch = np.asarray(inputs["batch"]).astype(np.int64)

    E = edge_index.shape[1]
    n_blocks = N_CORES * blocks_per_core
    nodes_pad = n_blocks * P
    assert nodes_pad >= n_nodes

    src, dst = edge_index[0], edge_index[1]
    indeg = np.bincount(dst, minlength=n_nodes)

    # LPT: sort nodes by in-degree desc, place into block with min edge load
    # among blocks that still have free slots.
    order = np.argsort(-indeg, kind="stable")
    block_load = np.zeros(n_blocks, np.int64)
    block_fill = np.zeros(n_blocks, np.int64)
    perm = np.full(n_nodes, -1, np.int64)  # old -> new id
    import heapq
    heap = [(0, 0, b) for b in range(n_blocks)]
    heapq.heapify(heap)
    for nd in order:
        while True:
            load, fill, b = heapq.heappop(heap)
            if block_fill[b] < P:
                break
        perm[nd] = b * P + block_fill[b]
        block_fill[b] += 1
        block_load[b] += indeg[nd]
        if block_fill[b] < P:
            heapq.heappush(heap, (block_load[b], block_fill[b], b))
    assert (perm >= 0).all()

    C_MAX = int(np.ceil(block_load.max() / P))
    C_MAX = (C_MAX + 3) // 4 * 4
    E_BLK = C_MAX * P

    new_src = perm[src]
    new_dst = perm[dst]
    dst_block = new_dst // P

    # order edges by dst block, arbitrary within
    eorder = np.argsort(dst_block, kind="stable")
    cfg = dict(C_MAX=C_MAX, E_BLK=E_BLK, blocks_per_core=blocks_per_core,
               nodes_pad=nodes_pad, n_blocks=n_blocks, n_graphs=n_graphs)

    # per-block padded edge arrays
    idx0_all = np.zeros((n_blocks, E_BLK), np.int16)   # atom id of src (layer 0)
    idxs_all = np.zeros((n_blocks, E_BLK), np.int16)   # new src id (layers 1-3)
    dstrel_all = np.full((n_blocks, E_BLK), -1.0, np.float32)
    ea_all = np.zeros((n_blocks, 8, E_BLK), np.float32)

    counts = np.bincount(dst_block[eorder], minlength=n_blocks)
    starts = np.concatenate([[0], np.cumsum(counts)])
    for b in range(n_blocks):
        es = eorder[starts[b]:starts[b + 1]]
        k = len(es)
        assert k <= E_BLK, (b, k, E_BLK)
        idx0_all[b, :k] = x_atoms[src[es]]
        idxs_all[b, :k] = new_src[es]
        dstrel_all[b, :k] = (new_dst[es] % P).astype(np.float32)
        ea_all[b, :4, :k] = edge_attr[es].T
        ea_all[b, 4, :k] = 1.0

    # pool data: node slot -> graph id and weight 1/cnt
    cnts = np.bincount(batch, minlength=n_graphs).astype(np.float32)
    inv_cnt = 1.0 / np.maximum(cnts, 1.0)
    bg = np.full(nodes_pad, -1.0, np.float32)
    wnode = np.zeros(nodes_pad, np.float32)
    bg[perm] = batch.astype(np.float32)
    wnode[perm] = inv_cnt[batch]

    def wrap16(a):  # [n] -> [128, n//16] wrap + replicate
        n = a.shape[-1]
        t = a.reshape(*a.shape[:-1], n // 16, 16)
        t = np.swapaxes(t, -1, -2)  # [..., 16, n//16]
        return np.tile(t, (1,) * (a.ndim - 1) + (8, 1))

    per_core = []
    B = blocks_per_core
    for c in range(N_CORES):
        sl = slice(c * B, (c + 1) * B)
        per_core.append(dict(
            idx0=wrap16(idx0_all[sl]),            # [B, 128, E_BLK//16]
            idxs=wrap16(idxs_all[sl]),            # [B, 128, E_BLK//16]
            idx0_pc=idx0_all[sl].astype(np.int32).reshape(B, C_MAX, P)
                .transpose(0, 2, 1).copy(),       # [B, 128, C]
            idxs_pc=idxs_all[sl].astype(np.int32).reshape(B, C_MAX, P)
                .transpose(0, 2, 1).copy(),
            dst_rel=dstrel_all[sl].reshape(B, C_MAX, P).transpose(0, 2, 1).copy(),  # [B,128,C]
            ea=ea_all[sl],                        # [B, 8, E_BLK]
            bg0=bg[c * B * P:(c + 1) * B * P].reshape(B, P, 1),
            bg1=(bg[c * B * P:(c + 1) * B * P].reshape(B, P, 1) - 128.0),
            wnode=wnode[c * B * P:(c + 1) * B * P].reshape(B, P, 1),
        ))
        # bg1: dummy (-1) becomes -129, still matches nothing in 0..127
    consts = dict(perm=perm, inv_cnt=inv_cnt, C_MAX=C_MAX)
    return cfg, per_core, consts


def prep_weights(inputs, d0=92):
    """Padded weight tensors (replicated to each core)."""
    f32 = np.float32
    emb = np.asarray(inputs["emb"]).astype(f32)       # [119, 92]
    emb_pad = np.zeros((P, P), f32)
    emb_pad[:emb.shape[0], :d0] = emb

    def ext(w, b, M):  # [4,M],[M] -> [8,M] rows 0-3 w, 4 bias
        o = np.zeros((8, M), f32)
        o[:4] = np.asarray(w, f32)
        o[4] = np.asarray(b, f32)
        return o

    W = {}
    W["emb_pad"] = emb_pad
    W["w1e_0"] = np.zeros((8, P), f32)
    W["w1e_0"][:4, :d0] = np.asarray(inputs["ew1_0"], f32)
    W["w1e_0"][4, :d0] = np.asarray(inputs["eb1_0"], f32)
    W["w2_0"] = np.zeros((P, P), f32)
    W["w2_0"][:d0, :d0] = np.asarray(inputs["ew2_0"], f32)
    W["b2rep_0"] = np.zeros((P, P), f32)
    W["b2rep_0"][:, :d0] = np.asarray(inputs["eb2_0"], f32)[None, :]
    W["nw_0"] = np.zeros((P, 256), f32)
    W["nw_0"][:d0] = np.asarray(inputs["nw_0"], f32)
    W["nbrep_0"] = np.tile(np.asarray(inputs["nb_0"], f32)[None, :], (P, 1))
    for l in range(3):
        W[f"w1e_{l+1}"] = ext(np.asarray(inputs["ew1"][l], f32),
                              np.asarray(inputs["eb1"][l], f32), 256)
        W[f"w2_{l+1}"] = np.asarray(inputs["ew2"][l], f32)        # [256,256]
        W[f"b2rep_{l+1}"] = np.tile(np.asarray(inputs["eb2"][l], f32)[None, :], (P, 1))
        W[f"nw_{l+1}"] = np.asarray(inputs["nw"][l], f32)         # [256,256]
        W[f"nbrep_{l+1}"] = np.tile(np.asarray(inputs["nb"][l], f32)[None, :], (P, 1))
    W["rw1"] = np.asarray(inputs["rw1"], f32)   # [256,256]
    W["rb1"] = np.asarray(inputs["rb1"], f32).reshape(256, 1)
    W["rw2"] = np.asarray(inputs["rw2"], f32)   # [256,128]
    W["rb2"] = np.asarray(inputs["rb2"], f32).reshape(128, 1)
    W["rw3"] = np.asarray(inputs["rw3"], f32).reshape(128, 1)
    W["rb3"] = np.asarray(inputs["rb3"], f32).reshape(1, 1)
    W["colidx"] = np.tile(np.arange(P, dtype=f32), (P, 1))
    return W


def numpy_forward(cfg, per_core, W, n_graphs):
    B, C, E_BLK = cfg["blocks_per_core"], cfg["C_MAX"], cfg["E_BLK"]
    nodes_pad = cfg["nodes_pad"]

    def unwrap(idx_t):
        n = idx_t.shape[-1] * 16
        return idx_t[:16].T.reshape(n).astype(np.int64)

    def run_layer(table, l, D):
        # D = feature width this layer operates on (92-padded-to-128 for l0)
        x_next = np.zeros((nodes_pad, 256), np.float32)
        for c in range(N_CORES):
            pc = per_core[c]
            key = "idx0" if l == 0 else "idxs"
            for b in range(B):
                ea = pc["ea"][b]
                if l == 0:
                    z = W["w1e_0"].T @ ea            # [128, E_BLK] (cols 92+ of w1e are 0)
                else:
                    z = W[f"w1e_{l}"].T @ ea         # [256, E_BLK]
                h = softplus(z)
                # NB: softplus(0)=log2!=0 on padded channels; w2 pad rows are 0 so ok
                w2 = W[f"w2_{l}"]
                if l == 0:
                    ee = h.T @ w2                    # [E_BLK, 128], pad cols 0
                else:
                    ee = h.T @ w2
                eeb = ee + W[f"b2rep_{l}"][0][None, :]
                idx = unwrap(pc[key][b])
                xg = table[idx]                      # [E_BLK, D]
                msg = eeb[:, :xg.shape[1]] * xg
                dst_rel = pc["dst_rel"][b]           # [128, C] p,j
                agg = np.zeros((P, msg.shape[1]), np.float32)
                for j in range(C):
                    for p in range(P):
                        d = int(dst_rel[p, j])
                        if d >= 0:
                            agg[d] += msg[j * P + p]
                nw = W[f"nw_{l}"]
                xb = softplus(agg @ nw[:agg.shape[1]] + W[f"nbrep_{l}"][0][None, :])
                x_next[(c * B + b) * P:(c * B + b + 1) * P] = xb
        return x_next

    x = run_layer(W["emb_pad"], 0, 128)
    for l in (1, 2, 3):
        x = run_layer(x, l, 256)

    # pool (weighted one-hot), graphs 0..255
    gsum = np.zeros((n_graphs, 256), np.float32)
    for c in range(N_CORES):
        pc = per_core[c]
        for b in range(B):
            xb = x[(c * B + b) * P:(c * B + b + 1) * P]
            bg = pc["bg0"][b][:, 0]
            w = pc["wnode"][b][:, 0]
            for p in range(P):
                g = int(bg[p])
                if g >= 0:
                    gsum[g] += w[p] * xb[p]
    h1 = softplus(gsum @ W["rw1"] + W["rb1"][:, 0][None, :])
    h2 = softplus(h1 @ W["rw2"] + W["rb2"][:, 0][None, :])
    out = (h2 @ W["rw3"])[:, 0] + W["rb3"][0, 0]
    return out





def _setup_act_tables(nc):
    """Reorder act_info.json so the set containing BOTH exp and ln comes
    first, and point bass AND walrus (BASS_ACT_ROOT_JSON_PATH) at the same
    file. Collapses per-block ACT table reloads to ~one total. Fail-safe:
    falls back to default tables on any error (correct, just slower)."""
    import os, json, functools
    import concourse.bacc as _bacc
    if getattr(_bacc, "_act_reordered", False):
        return
    try:
        from neuronxcc.driver.Job import Job
        from neuronxcc.driver.jobs.support.FindActInfo import findActInfoFile
        arch = nc.m.arch
        src_path = findActInfoFile(Job.getPackageDir(), arch)
        with open(src_path) as f:
            d = json.load(f)
        pref = "natural_log_exp_and_others"
        sets = d["act_func_sets"]
        if pref not in [e["name"] for e in sets]:
            return
        d["act_func_sets"] = ([e for e in sets if e["name"] == pref]
                              + [e for e in sets if e["name"] != pref])
        path = "/tmp/act_info_reordered_%s.json" % arch
        with open(path, "w") as f:
            json.dump(d, f)
        os.environ["BASS_ACT_ROOT_JSON_PATH"] = path

        @functools.cache
        def patched(a):
            with open(path) as f:
                info = json.load(f)
            return {e["name"]: {mybir.ActivationFunctionType.from_pwp(v)
                                for v in e["act"].keys()}
                    for e in info["act_func_sets"]}

        _bacc.get_activation_tables = patched
        _bacc._act_reordered = True
    except Exception:
        pass


def build(cfg):
    B = cfg["blocks_per_core"]
    C = cfg["C_MAX"]
    E_BLK = C * P
    E16 = E_BLK // 16
    nodes_pad = cfg["nodes_pad"]
    assert nodes_pad == N_CORES * B * P

    nc = bacc.Bacc("TRN2", target_bir_lowering=False, debug=False,
                   num_devices=N_CORES)
    # NOTE: reordering act_info.json via BASS_ACT_ROOT_JSON_PATH collapses
    # the 364 ACT table reloads to 1, but the resulting NEFF fails
    # LoadExecutable on this terminal runtime — do not enable.

    # ---- I/O ----
    t_in = {}
    def din(name, shape, dt=F32):
        t_in[name] = nc.dram_tensor(name, shape, dt, kind="ExternalInput")
        return t_in[name]

    din("idx0", [P, B * E16], I16)
    din("idxs", [P, B * E16], I16)
    din("dstrel", [P, B * C])
    din("ea", [B, 8, E_BLK])
    din("bg0", [P, B]); din("bg1", [P, B]); din("wnode", [P, B])
    din("emb", [P, P])            # gather table for layer 0
    din("w1e_0", [8, P]); din("w2_0", [P, P]); din("b2rep2_0", [P, 2 * P])
    din("nw_0", [P, D_HID]); din("nbrep_0", [P, D_HID])
    for l in (1, 2, 3):
        din(f"w1e_{l}", [8, D_HID]); din(f"w2_{l}", [P, 2 * D_HID])
        din(f"b2rep2_{l}", [P, 2 * D_HID]); din(f"nw_{l}", [P, 2 * D_HID])
        din(f"nbrep_{l}", [P, D_HID])
    din("rw1", [P, 512]); din("rb1", [P, 2]); din("rw2", [P, 256])
    din("rb2", [P, 1]); din("rw3", [P, 1]); din("rb3", [1, 1])
    din("colidx", [P, P])
    out_t = nc.dram_tensor("out", [1, NG], F32, kind="ExternalOutput")

    with tile.TileContext(nc) as tc:
        with (
            tc.tile_pool(name="const", bufs=1) as cpool,
            tc.tile_pool(name="work", bufs=2) as wpool,
            tc.tile_pool(name="work3", bufs=3) as w3pool,
            tc.tile_pool(name="psum", bufs=1, space="PSUM") as pspool,
            tc.tile_pool(name="psum2", bufs=2, space="PSUM") as ps2pool,
            tc.tile_pool(name="dram", bufs=1, space="DRAM") as dpool,
        ):
            nc.gpsimd.load_library(library_config.mlp)

            # ---- load constants/weights into SBUF ----
            ct = {}
            for name, h in t_in.items():
                if name in ("ea", "emb"):
                    continue
                dt = I16 if name in ("idx0", "idxs") else F32
                ct[name] = cpool.tile(list(h.shape), dt, tag=f"c_{name}", name=f"c_{name}")
                nc.sync.dma_start(ct[name][:], h[:])
            ident = cpool.tile([P, P], F32, tag="ident")
            make_identity(nc, ident[:])
            cb = cpool.tile([P, C * P], F32, tag="cb")
            for j in range(C):
                nc.sync.dma_start(cb[:, j * P:(j + 1) * P], t_in["colidx"][:])

            # DRAM bounce buffers for collectives
            x_loc = [dpool.tile([B * P, D_HID], F32, name=f"x_loc{i}") for i in range(3)]
            x_glob = [dpool.tile([nodes_pad, D_HID], F32, name=f"x_glob{i}") for i in range(3)]
            gr_in = dpool.tile([2 * P, NG], F32)
            gr_out = dpool.tile([2 * P, NG], F32)

            gacc = cpool.tile([P, 2 * NG], F32, tag="gacc")
            nc.vector.memset(gacc[:], 0.0)

            import os as _os
            _NL = int(_os.environ.get("GNN_LAYERS", "4"))
            for l in range(_NL):
                D_in = P if l == 0 else D_HID
                KT = 1 if l == 0 else 2
                lw = "0" if l == 0 else str(l)
                w1e = ct[f"w1e_{lw}"]
                w2 = ct[f"w2_{lw}"]
                b2rep2 = ct[f"b2rep2_{lw}"]
                nw = ct[f"nw_{lw}"]
                nbrep = ct[f"nbrep_{lw}"]
                idx_t = ct["idx0"] if l == 0 else ct["idxs"]
                gsrc = t_in["emb"] if l == 0 else x_glob[l - 1]

                def flush_node_softplus(pb, pxb):
                    # deferred from block pb: Ln + guard + store/pool
                    xs = wpool.tile([P, D_HID], F32, tag="xs", name="xs")
                    nc.scalar.activation(xs[:], pend_xe[0][:], AF.Ln, bias=1.0)
                    if l == 3:
                        xm = wpool.tile([P, D_HID], mybir.dt.uint8, tag="xm",
                                        name="xm")
                        nc.vector.tensor_scalar(
                            out=xm[:], in0=pxb[:], scalar1=25.0, scalar2=None,
                            op0=OP.is_gt)
                        nc.vector.copy_predicated(out=xs[:], mask=xm[:],
                                                  data=pxb[:])
                    if l < 3:
                        nc.sync.dma_start(x_loc[l][pb * P:(pb + 1) * P, :],
                                          xs[:])
                    elif _os.environ.get("GNN_POOL", "1") == "1":
                        ohp = wpool.tile([P, NG], F32, tag="ohp", name="ohp")
                        for gh, bgk in ((0, "bg0"), (1, "bg1")):
                            nc.vector.tensor_scalar(
                                out=ohp[:, gh * P:(gh + 1) * P],
                                in0=ct["colidx"][:],
                                scalar1=ct[bgk][:, pb:pb + 1],
                                scalar2=ct["wnode"][:, pb:pb + 1],
                                op0=OP.is_equal, op1=OP.mult)
                        for ch in range(2):
                            pp = pspool.tile([P, NG], F32, tag=f"pool{ch}",
                                             space="PSUM", name=f"pp{ch}")
                            nc.tensor.matmul(
                                pp[:],
                                lhsT=xs[:, ch * P:(ch + 1) * P],
                                rhs=ohp[:],
                                start=True, stop=True)
                            nc.vector.tensor_tensor(
                                out=gacc[:, ch * NG:(ch + 1) * NG],
                                in0=gacc[:, ch * NG:(ch + 1) * NG],
                                in1=pp[:], op=OP.add)

                pend = [None]   # (b, xb tile) awaiting Exp
                pend_xe = [None]  # xe tile awaiting Ln
                for b in range(B):
                    # gather x[src] for this block: [128, C, D_in]
                    # dma_gather fails above ~1024 idxs; 512-idx sub-gathers
                    xg = w3pool.tile([P, C, D_in], F32, tag="xg")
                    for s in range(C // 4):
                        nc.gpsimd.dma_gather(
                            xg[:, 4 * s:4 * s + 4, :], gsrc[:],
                            idx_t[:, b * E16 + 32 * s: b * E16 + 32 * (s + 1)],
                            512, 512, D_in)

                    # edge attrs
                    ea_t = wpool.tile([8, E_BLK], F32, tag="ea")
                    nc.sync.dma_start(ea_t[:], t_in["ea"][b])

                    # one-hot scatter matrix [128, C*128], one DVE op
                    oh = wpool.tile([P, E_BLK], F32, tag="oh")
                    nc.vector.tensor_tensor(
                        out=oh[:], in0=cb[:],
                        in1=ct["dstrel"][:, b * C:(b + 1) * C]
                            .to_broadcast([P, C, P]),
                        op=OP.is_equal)

                    # z^T / h^T channel-major; z psum per 512-edge subchunk
                    # (single PSUM bank per matmul/ACT access)
                    h_t = wpool.tile([P, KT * E_BLK], F32, tag="h")
                    e_t = wpool.tile([P, KT * E_BLK], F32, tag="et")
                    for m in range(KT):
                        for s in range(E_BLK // 512):
                            z_ps = ps2pool.tile([P, 512], F32, tag="z",
                                                space="PSUM")
                            nc.tensor.matmul(
                                z_ps[:],
                                lhsT=w1e[:, m * P:(m + 1) * P],
                                rhs=ea_t[:, s * 512:(s + 1) * 512],
                                start=True, stop=True)
                            nc.scalar.activation(
                                e_t[:, m * E_BLK + s * 512:
                                    m * E_BLK + (s + 1) * 512],
                                z_ps[:], AF.Exp)
                    if pend[0] is not None:
                        xe = wpool.tile([P, D_HID], F32, tag="xe", name="xe")
                        nc.scalar.activation(xe[:], pend[0][1][:], AF.Exp)
                        pend_xe[0] = xe
                    for m in range(KT):
                        nc.scalar.activation(
                            h_t[:, m * E_BLK:(m + 1) * E_BLK],
                            e_t[:, m * E_BLK:(m + 1) * E_BLK],
                            AF.Ln, bias=1.0)
                    if pend[0] is not None:
                        flush_node_softplus(pend[0][0], pend[0][1])
                        pend[0] = None

                    # per 2-chunk group: ee -> +b2 -> *xg -> scatter-matmul
                    agg_ps = ps2pool.tile([P, D_in], F32, tag="aggc", space="PSUM")
                    for g in range(C // 2):
                        ee_ps = pspool.tile([P, 2 * D_in], F32, tag="ee",
                                            space="PSUM")
                        for j2 in range(2):
                            j = 2 * g + j2
                            for k in range(KT):
                                nc.tensor.matmul(
                                    ee_ps[:, j2 * D_in:(j2 + 1) * D_in],
                                    lhsT=h_t[:, k * E_BLK + j * P:
                                             k * E_BLK + (j + 1) * P],
                                    rhs=w2[:, k * D_in:(k + 1) * D_in],
                                    start=(k == 0), stop=(k == KT - 1))
                        eeb = wpool.tile([P, 2 * D_in], F32, tag="eeb")
                        nc.vector.tensor_tensor(
                            out=eeb[:], in0=ee_ps[:], in1=b2rep2[:, :2 * D_in],
                            op=OP.add)
                        msg = wpool.tile([P, 2 * D_in], F32, tag="msg")
                        nc.vector.tensor_tensor(
                            out=msg[:], in0=eeb[:],
                            in1=xg[:, 2 * g:2 * g + 2, :],
                            op=OP.mult)
                        for j2 in range(2):
                            j = 2 * g + j2
                            nc.tensor.matmul(
                                agg_ps[:],
                                lhsT=oh[:, j * P:(j + 1) * P],
                                rhs=msg[:, j2 * D_in:(j2 + 1) * D_in],
                                start=(j == 0), stop=(j == C - 1))

                    # agg -> aggT (PE transpose), node linear, softplus
                    agg_sb = wpool.tile([P, D_in], F32, tag="aggsb")
                    nc.vector.tensor_copy(out=agg_sb[:], in_=agg_ps[:])
                    aggT_ps = ps2pool.tile([P, D_in], F32, tag="aggc", space="PSUM")
                    for k in range(KT):
                        nc.tensor.transpose(
                            aggT_ps[:, k * P:(k + 1) * P],
                            agg_sb[:, k * P:(k + 1) * P], ident[:])
                    aggT_sb = wpool.tile([P, D_in], F32, tag="aggTsb")
                    nc.vector.tensor_copy(out=aggT_sb[:], in_=aggT_ps[:])

                    xp_ps = ps2pool.tile([P, D_HID], F32, tag="aggc", space="PSUM")
                    for k in range(KT):
                        nc.tensor.matmul(
                            xp_ps[:], lhsT=aggT_sb[:, k * P:(k + 1) * P],
                            rhs=nw[:, k * D_HID:(k + 1) * D_HID],
                            start=(k == 0), stop=(k == KT - 1))
                    xb = wpool.tile([P, D_HID], F32, tag="xb")
                    nc.vector.tensor_tensor(out=xb[:], in0=xp_ps[:],
                                            in1=nbrep[:], op=OP.add)
                    pend[0] = (b, xb)

                # flush final block of this layer
                xe = wpool.tile([P, D_HID], F32, tag="xe", name="xe_f")
                nc.scalar.activation(xe[:], pend[0][1][:], AF.Exp)
                pend_xe[0] = xe
                flush_node_softplus(pend[0][0], pend[0][1])
                pend[0] = None

                if l < 3:
                    nc.gpsimd.collective_compute(
                        "AllGather", OP.bypass,
                        ins=[x_loc[l].opt()], outs=[x_glob[l].opt()],
                        replica_groups=[list(range(N_CORES))])

            # ---- pool partials -> AllReduce -> readout MLP ----
            if _NL < 4 or _os.environ.get("GNN_RO", "1") == "0":
                dummy = cpool.tile([1, NG], F32, tag="dummy")
                nc.vector.memset(dummy[:], 0.0)
                nc.sync.dma_start(out_t[:], dummy[:])
            else:
                for ch in range(2):
                    nc.sync.dma_start(gr_in[ch * P:(ch + 1) * P, :],
                                      gacc[:, ch * NG:(ch + 1) * NG])
                nc.gpsimd.collective_compute(
                    "AllReduce", OP.add, ins=[gr_in.opt()], outs=[gr_out.opt()],
                    replica_groups=[list(range(N_CORES))])
                gmT = cpool.tile([P, 2 * NG], F32, tag="gmT")
                for k in range(2):
                    nc.sync.dma_start(gmT[:, k * NG:(k + 1) * NG],
                                      gr_out[k * P:(k + 1) * P, :])

                h1T = cpool.tile([P, 2 * NG], F32, tag="h1T")
                for m in range(2):
                    h1_ps = pspool.tile([P, NG], F32, tag=f"pool{m}", space="PSUM",
                                        name=f"h1_ps{m}")
                    for k in range(2):
                        nc.tensor.matmul(
                            h1_ps[:],
                            lhsT=ct["rw1"][:, k * NG + m * P: k * NG + (m + 1) * P],
                            rhs=gmT[:, k * NG:(k + 1) * NG],
                            start=(k == 0), stop=(k == 1))
                    h1e = cpool.tile([P, NG], F32, tag="h1e", name=f"h1e{m}")
                    nc.scalar.activation(h1e[:], h1_ps[:], AF.Exp,
                                         bias=ct["rb1"][:, m:m + 1])
                    nc.scalar.activation(h1T[:, m * NG:(m + 1) * NG], h1e[:],
                                         AF.Ln, bias=1.0)
                h2_ps = pspool.tile([P, NG], F32, tag="pool0", space="PSUM")
                for k in range(2):
                    nc.tensor.matmul(
                        h2_ps[:], lhsT=ct["rw2"][:, k * P:(k + 1) * P],
                        rhs=h1T[:, k * NG:(k + 1) * NG],
                        start=(k == 0), stop=(k == 1))
                h2e = cpool.tile([P, NG], F32, tag="h2e")
                nc.scalar.activation(h2e[:], h2_ps[:], AF.Exp,
                                     bias=ct["rb2"][:, :1])
                h2T = cpool.tile([P, NG], F32, tag="h2T")
                nc.scalar.activation(h2T[:], h2e[:], AF.Ln, bias=1.0)
                o_ps = pspool.tile([1, NG], F32, tag="pool1", space="PSUM")
                nc.tensor.matmul(o_ps[:], lhsT=ct["rw3"][:, :1], rhs=h2T[:],
                                 start=True, stop=True)
                o_sb = cpool.tile([1, NG], F32, tag="osb")
                nc.scalar.activation(o_sb[:], o_ps[:], AF.Identity,
                                     bias=ct["rb3"][:, :1])
                nc.sync.dma_start(out_t[:], o_sb[:])


    nc.compile()
    return nc


def make_in_maps(cfg, per_core, W):
    """Assemble per-core input dicts matching build()'s tensor names."""
    B, C = cfg["blocks_per_core"], cfg["C_MAX"]
    E_BLK = C * P
    E16 = E_BLK // 16
    maps = []
    for c in range(N_CORES):
        pc = per_core[c]
        m = dict(
            idx0=pc["idx0"].transpose(1, 0, 2).reshape(P, B * E16).copy(),
            idxs=pc["idxs"].transpose(1, 0, 2).reshape(P, B * E16).copy(),
            dstrel=pc["dst_rel"].transpose(1, 0, 2).reshape(P, B * C).copy(),
            ea=pc["ea"].copy(),
            bg0=pc["bg0"][:, :, 0].T.copy(), bg1=pc["bg1"][:, :, 0].T.copy(),
            wnode=pc["wnode"][:, :, 0].T.copy(),
            emb=W["emb_pad"], w1e_0=W["w1e_0"], w2_0=W["w2_0"],
            b2rep2_0=np.concatenate([W["b2rep_0"]] * 2, 1),
            nw_0=W["nw_0"], nbrep_0=W["nbrep_0"],
            rw1=np.concatenate([W["rw1"][:P], W["rw1"][P:]], 1),
            rb1=np.concatenate([W["rb1"][:P], W["rb1"][P:]], 1),
            rw2=np.concatenate([W["rw2"][:P], W["rw2"][P:]], 1),
            rb2=W["rb2"], rw3=W["rw3"], rb3=W["rb3"], colidx=W["colidx"],
        )
        for l in (1, 2, 3):
            m[f"w1e_{l}"] = W[f"w1e_{l}"]
            m[f"w2_{l}"] = np.concatenate(
                [W[f"w2_{l}"][:P], W[f"w2_{l}"][P:]], 1)
            m[f"b2rep2_{l}"] = np.concatenate([W[f"b2rep_{l}"]] * 2, 1)
            m[f"nw_{l}"] = np.concatenate(
                [W[f"nw_{l}"][:P], W[f"nw_{l}"][P:]], 1)
            m[f"nbrep_{l}"] = W[f"nbrep_{l}"]
        maps.append({k: np.ascontiguousarray(v) for k, v in m.items()})
    return maps


_CACHE = {}


def _get_compiled(cfg):
    key = (cfg["C_MAX"], cfg["blocks_per_core"])
    if key not in _CACHE:
        _CACHE[key] = build(cfg)
    return _CACHE[key]


def kernel(**inputs) -> np.ndarray:
    cfg, per_core, consts = prep(inputs, N_NODES, N_GRAPHS, blocks_per_core=25)
    W = prep_weights(inputs)
    nc = _get_compiled(cfg)
    in_maps = make_in_maps(cfg, per_core, W)
    res = run_bass_kernel_spmd(nc, in_maps, core_ids=list(range(N_CORES)))
    return np.asarray(res.results[0]["out"][0], dtype=np.float32)


def _run_repeat(nc, in_maps, iters=10):
    """Execute the compiled kernel repeatedly with device-resident inputs;
    returns (results_core0, per-iter seconds list). Mirrors
    bass2jax.run_bass_via_pjrt's multi-core path without donation."""
    import time
    import jax
    import jax.numpy as jnp
    from jax.experimental.shard_map import shard_map
    from jax.sharding import Mesh, PartitionSpec
    jax.devices()  # init backend BEFORE bass2jax import so the "neuron"
    # platform is known when bass2jax registers its mlir lowering
    from concourse import bass2jax
    from concourse import mybir as _mybir

    bass2jax.install_neuronx_cc_hook()
    n_cores = len(in_maps)
    partition_name = (nc.partition_id_tensor.name
                      if nc.partition_id_tensor else None)
    in_names, out_names, out_avals, zero_outs = [], [], [], []
    for alloc in nc.m.functions[0].allocations:
        if not isinstance(alloc, _mybir.MemoryLocationSet):
            continue
        name = alloc.memorylocations[0].name
        if alloc.kind == "ExternalInput":
            if name != partition_name:
                in_names.append(name)
        elif alloc.kind == "ExternalOutput":
            shape = tuple(alloc.tensor_shape)
            dtype = _mybir.dt.np(alloc.dtype)
            out_names.append(name)
            out_avals.append(jax.core.ShapedArray(shape, dtype))
            zero_outs.append(np.zeros(shape, dtype))
    n_params = len(in_names)
    all_in_names = list(in_names) + list(out_names)
    if partition_name is not None:
        all_in_names.append(partition_name)

    def _body(*args):
        operands = list(args)
        if partition_name is not None:
            operands.append(bass2jax.partition_id_tensor())
        outs = bass2jax._bass_exec_p.bind(
            *operands,
            out_avals=tuple(out_avals),
            in_names=tuple(all_in_names),
            out_names=tuple(out_names),
            lowering_input_output_aliases=(),
            sim_require_finite=True,
            sim_require_nnan=True,
            nc=nc,
        )
        return tuple(outs)

    devices = jax.devices()[:n_cores]
    mesh = Mesh(np.asarray(devices), ("core",))
    in_specs = (PartitionSpec("core"),) * (n_params + len(out_names))
    out_specs = (PartitionSpec("core"),) * len(out_names)
    fn = jax.jit(shard_map(_body, mesh=mesh, in_specs=in_specs,
                           out_specs=out_specs, check_rep=False),
                 keep_unused=True)
    concat_in = [np.concatenate([np.asarray(in_maps[c][n]) for c in range(n_cores)], 0)
                 for n in in_names]
    concat_zeros = [np.zeros((n_cores * z.shape[0], *z.shape[1:]), z.dtype)
                    for z in zero_outs]
    sharding = jax.sharding.NamedSharding(mesh, PartitionSpec("core"))
    dev_args = [jax.device_put(a, sharding) for a in concat_in + concat_zeros]
    outs = fn(*dev_args)
    jax.block_until_ready(outs)
    times = []
    for _ in range(iters):
        t0 = time.perf_counter()
        outs = fn(*dev_args)
        jax.block_until_ready(outs)
        times.append(time.perf_counter() - t0)
    res0 = {n: np.asarray(outs[i]).reshape(n_cores, *out_avals[i].shape)[0]
            for i, n in enumerate(out_names)}
    return res0, times


def bench(iters=10, **inputs):
    cfg, per_core, consts = prep(inputs, N_NODES, N_GRAPHS, blocks_per_core=25)
    W = prep_weights(inputs)
    nc = _get_compiled(cfg)
    in_maps = make_in_maps(cfg, per_core, W)
    res0, times = _run_repeat(nc, in_maps, iters)
    return np.asarray(res0["out"][0], dtype=np.float32), times

